# revision 1
# baseline (speedup 1.0000x reference)
"""Multi-head attention (B=4, T=2048, D=768, H=12) on 8 NeuronCores.

Sharding: core c handles batch b = c//2 and head-group g = c%2 (heads
6g..6g+5).  Each core computes its 6 heads' attention and a partial
output projection (contraction over its 384 local dims of w_proj).  The
host sums the two partials per batch and adds the bias terms.

Device-side formulation (everything transposed so the contraction dim
always lands on SBUF partitions):
  xT   [768, 2048]  (host pre-transposes x[b])
  qT   = Wq_loc.T @ xT   [384, 2048]   (scaled by 1/sqrt(hd), +bias)
  kT   = Wk_loc.T @ xT   [384, 2048]   (+bias)
  v    = x @ Wv_loc      [2048, 384]   (normal layout, no bias; the
         v-bias contributes a constant row handled on host)
  S^T  = kT_h.T @ qT_h   [kpos, q]  per head
  P^T  = exp(S^T)        (scores max ~8 -> no max subtraction needed)
  O'^T = [v_h | 1]^T @ P^T  [65, q]  accumulated over kpos tiles;
         row 64 = softmax denominators
  O^T  normalized via E-matmul broadcast of reciprocal denominators
  y    = O_loc @ Wp_loc  [2048, 768]  partial (host adds partner core)

Schedule: input DMAs are chunked so PE starts within a few us; the V
projection is fused per-kpos-tile into head 0's attention sweep; the
QKV projection of pair p+1 fills PE gaps while ACT paces attention of
pair p; the dt<2 half of the output projection runs during the last
head pair's attention.
"""

import numpy as np

EMBED = 768
HEADS = 12
HD = 64
SCALE = HD ** -0.5
B, T = 4, 2048
NCORES = 8
HPC = 6            # heads per core
DL = HPC * HD      # 384 local model dims per core
USE_FP32R = True

_prog_cache = {}


def _build_program(repeat=1):
    import concourse.bass as bass
    import concourse.mybir as mybir
    import concourse.tile as tile
    from concourse import bacc

    f32 = mybir.dt.float32
    f32r = mybir.dt.float32r
    ACT_EXP = mybir.ActivationFunctionType.Exp

    fm = f32r if USE_FP32R else f32   # storage dtype of matmul operands

    nc = bacc.Bacc()

    xt_d = nc.dram_tensor("xt", [EMBED, T], fm, kind="ExternalInput")
    wq_d = nc.dram_tensor("wq", [EMBED, DL], fm, kind="ExternalInput")
    wk_d = nc.dram_tensor("wk", [EMBED, DL], fm, kind="ExternalInput")
    wv_d = nc.dram_tensor("wv", [EMBED, DL], fm, kind="ExternalInput")
    bqs_d = nc.dram_tensor("bqs", [DL], f32, kind="ExternalInput")
    bk_d = nc.dram_tensor("bk", [DL], f32, kind="ExternalInput")
    wp_d = nc.dram_tensor("wp", [DL, EMBED], fm, kind="ExternalInput")
    e2_d = nc.dram_tensor("e2", [2, 128], fm, kind="ExternalInput")
    y_d = nc.dram_tensor("y", [T, EMBED], f32, kind="ExternalOutput")

    NDT = EMBED // 128   # 6 contraction tiles over embed dim
    NKT = T // 128       # 16 key-position tiles
    NQT = T // 128       # 16 query row tiles
    QH = 2               # process queries in halves of 1024
    QHW = T // QH        # 1024

    with tile.TileContext(nc) as tc:
      for _rep in range(repeat):
        with tc.tile_pool(name="persist", bufs=1) as pers, \
             tc.tile_pool(name="qk", bufs=2) as qk_pool, \
             tc.tile_pool(name="r6p", bufs=2) as r6_pool, \
             tc.tile_pool(name="ps", bufs=2, space="PSUM") as ps_pool, \
             tc.tile_pool(name="ps_s", bufs=2, space="PSUM") as pss_pool, \
             tc.tile_pool(name="ps_o", bufs=2, space="PSUM") as pso_pool, \
             tc.tile_pool(name="pT", bufs=3) as pT_pool:
            v_sb = pers.tile([128, NKT, HPC, HD + 1], fm, name="v_sb")
            oT_sb = pers.tile([128, 3, T], fm, name="oT_sb")
            e2_sb = pers.tile([2, 128], fm, name="e2_sb")
            bqs_sb = pers.tile([128, 3], f32, name="bqs_sb")
            bk_sb = pers.tile([128, 3], f32, name="bk_sb")

            nc.sync.dma_start(out=e2_sb, in_=e2_d.ap())
            nc.gpsimd.dma_start(out=bqs_sb, in_=bqs_d.ap().rearrange("(n p) -> p n", p=128))
            nc.gpsimd.dma_start(out=bk_sb, in_=bk_d.ap().rearrange("(n p) -> p n", p=128))

            # ones column of v' (softmax denominator accumulator): fill the
            # whole tile with 1.0; the value copies below overwrite cols
            # 0:64 of each head slot, leaving col 64 = 1.0
            nc.gpsimd.memset(v_sb.bitcast(f32), 1.0)

            # PE warm-up while the input DMAs stream: dependency-free
            # matmuls on e2 ramp the PE power state before real work lands
            warm_sb = pers.tile([128, 512], fm, name="warm_sb")
            nc.vector.memset(warm_sb.bitcast(f32), 0.0)
            for wi in range(16):
                psw = ps_pool.tile([128, 512], f32, name="psw", tag="ps")
                nc.tensor.matmul(psw, warm_sb[0:2, 0:128], warm_sb[0:2, :],
                                 start=True, stop=True)

            qk_tiles = {}
            r6_tiles = {}

            def proj_qk_chunk(hp, ch, xt_sb, wq_sb, wk_sb):
                qTp, kTp = qk_tiles[hp]
                if True:
                    csl = bass.ts(ch, 512)
                    psq = ps_pool.tile([128, 512], f32, name="psq", tag="ps")
                    psk = ps_pool.tile([128, 512], f32, name="psk", tag="ps")
                    for dt in range(NDT):
                        nc.tensor.matmul(
                            psq,
                            wq_sb[:, dt, bass.ts(hp, 128)],
                            xt_sb[:, dt, csl],
                            start=(dt == 0), stop=(dt == NDT - 1),
                        )
                    for dt in range(NDT):
                        nc.tensor.matmul(
                            psk,
                            wk_sb[:, dt, bass.ts(hp, 128)],
                            xt_sb[:, dt, csl],
                            start=(dt == 0), stop=(dt == NDT - 1),
                        )
                    nc.vector.tensor_scalar(
                        out=qTp[:, csl], in0=psq,
                        scalar1=bqs_sb[:, hp:hp + 1], scalar2=float(SCALE),
                        op0=mybir.AluOpType.add, op1=mybir.AluOpType.mult,
                    )
                    nc.vector.tensor_scalar_add(
                        out=kTp[:, csl], in0=psk,
                        scalar1=bk_sb[:, hp:hp + 1],
                    )

            def proj_qk(hp, xt_sb, wq_sb, wk_sb):
                # qT/kT for head pair hp ([128, T] each, 2 heads stacked)
                qTp = qk_pool.tile([128, T], fm, name="qTp", tag="qT")
                kTp = qk_pool.tile([128, T], fm, name="kTp", tag="kT")
                qk_tiles[hp] = (qTp, kTp)
                for ch in range(4):
                    proj_qk_chunk(hp, ch, xt_sb, wq_sb, wk_sb)

            def emit_v(kt, xt_sb, wv_sb):
                # v (normal layout) for all 6 heads at kpos tile kt
                psv = ps_pool.tile([128, DL], f32, name="psv", tag="ps")
                for dt in range(NDT):
                    nc.tensor.matmul(
                        psv,
                        xt_sb[:, dt, bass.ts(kt, 128)],
                        wv_sb[:, dt, :],
                        start=(dt == 0), stop=(dt == NDT - 1),
                    )
                nc.vector.tensor_copy(
                    out=v_sb[:, kt, :, 0:HD],
                    in_=psv.rearrange("p (h d) -> p h d", h=HPC),
                )

            def attend(h, fuse_v=None, qhs=None):
                # one head: S^T -> exp -> O'^T, denominators to r6.
                # fuse_v: (xt_sb, wv_sb) to emit the V projection per kt
                # during the qh==0 sweep.
                hp, off = h // 2, (h % 2) * 64
                qTp, kTp = qk_tiles[hp]
                if h % 2 == 0 and hp not in r6_tiles:
                    r6_tiles[hp] = r6_pool.tile([2, T], fm, name="r6p", tag="r6")
                r6p = r6_tiles[hp]
                NC2 = QHW // 512
                for qh in (range(QH) if qhs is None else qhs):
                    psos = [
                        pso_pool.tile([65, 512], f32, name="pso", tag="pso")
                        for _ in range(NC2)
                    ]
                    for kt in range(NKT):
                        if fuse_v is not None and qh == 0:
                            emit_v(kt, *fuse_v)
                        pss = pss_pool.tile([128, QHW], f32, name="pss", tag="pss")
                        pT = pT_pool.tile([128, QHW], fm, name="pT", tag="pT")
                        for c2 in range(NC2):
                            nc.tensor.matmul(
                                pss[:, bass.ts(c2, 512)],
                                kTp[off:off + 64, bass.ts(kt, 128)],
                                qTp[off:off + 64, bass.ds(qh * QHW + c2 * 512, 512)],
                                start=True, stop=True,
                            )
                        nc.scalar.activation(out=pT, in_=pss, func=ACT_EXP)
                        for c2 in range(NC2):
                            nc.tensor.matmul(
                                psos[c2],
                                v_sb[:, kt, h, :],
                                pT[:, bass.ts(c2, 512)],
                                start=(kt == 0), stop=(kt == NKT - 1),
                            )
                    for c2 in range(NC2):
                        qssl = bass.ds(qh * QHW + c2 * 512, 512)
                        # engine ops need partition base in {0,32,64,96}:
                        # reciprocal at partition 64, then DMA the row down
                        # to r6's row for this head (DMA has no such limit)
                        rcp_sb = pT_pool.tile([65, 512], fm, name="rcp_sb", tag="rcp", bufs=2)
                        with nc.allow_low_precision(reason="fp32r storage"):
                            nc.vector.reciprocal(
                                out=rcp_sb[64:65, :], in_=psos[c2][64:65, :],
                            )
                        nc.sync.dma_start(
                            out=r6p[h % 2:h % 2 + 1, qssl], in_=rcp_sb[64:65, :],
                        )
                        nc.vector.tensor_copy(
                            out=oT_sb[off:off + 64, hp, qssl], in_=psos[c2][0:64, :],
                        )

            def normalize(hp):
                # oT[:, hp] *= broadcast(1/denom) via the E matmul,
                # chunked so psr rides the pss psum slots (no extra banks)
                r6p = r6_tiles[hp]
                for ch in range(4):
                    csl = bass.ts(ch, 512)
                    psr = ps_pool.tile([128, 512], f32, name="psr", tag="ps")
                    nc.tensor.matmul(
                        psr, e2_sb, r6p[:, csl],
                        start=True, stop=True,
                    )
                    nc.vector.tensor_mul(
                        out=oT_sb[:, hp, csl], in0=oT_sb[:, hp, csl], in1=psr,
                    )

            with tc.tile_pool(name="xw", bufs=1) as xw:
                xt_sb = xw.tile([128, NDT, T], fm, name="xt_sb")
                wq_sb = xw.tile([128, NDT, DL], fm, name="wq_sb")
                wk_sb = xw.tile([128, NDT, DL], fm, name="wk_sb")
                wv_sb = xw.tile([128, NDT, DL], fm, name="wv_sb")

                # chunked input DMAs: xt on the HWDGE queue, weights on the
                # SWDGE queue so they don't serialize behind xt
                nc.gpsimd.dma_start(out=wq_sb, in_=wq_d.ap().rearrange("(n p) m -> p n m", p=128))
                nc.gpsimd.dma_start(out=wk_sb, in_=wk_d.ap().rearrange("(n p) m -> p n m", p=128))
                for dt in range(NDT):
                    nc.sync.dma_start(
                        out=xt_sb[:, dt, :], in_=xt_d.ap()[bass.ts(dt, 128), :],
                    )
                nc.gpsimd.dma_start(out=wv_sb, in_=wv_d.ap().rearrange("(n p) m -> p n m", p=128))

                # startup: interleave qk-pair-0 chunks with head-0/qh0
                # attention steps (S needs only k-chunk kt//4 and q-chunk 0)
                qTp0 = qk_pool.tile([128, T], fm, name="qTp0", tag="qT")
                kTp0 = qk_pool.tile([128, T], fm, name="kTp0", tag="kT")
                qk_tiles[0] = (qTp0, kTp0)
                r6_tiles[0] = r6_pool.tile([2, T], fm, name="r6p0", tag="r6")
                pso0s = [
                    pso_pool.tile([65, 512], f32, name="pso0", tag="pso")
                    for _ in range(2)
                ]
                proj_qk_chunk(0, 0, xt_sb, wq_sb, wk_sb)
                for ch in range(1, 4):
                    proj_qk_chunk(0, ch, xt_sb, wq_sb, wk_sb)
                    for kt in range(4 * (ch - 1), 4 * (ch - 1) + (8 if ch == 3 else 4)):
                        emit_v(kt, xt_sb, wv_sb)
                        pss = pss_pool.tile([128, QHW], f32, name="pss", tag="pss")
                        pT = pT_pool.tile([128, QHW], fm, name="pT", tag="pT")
                        for c2 in range(2):
                            nc.tensor.matmul(
                                pss[:, bass.ts(c2, 512)],
                                kTp0[0:64, bass.ts(kt, 128)],
                                qTp0[0:64, bass.ts(c2, 512)],
                                start=True, stop=True,
                            )
                        nc.scalar.activation(out=pT, in_=pss, func=ACT_EXP)
                        for c2 in range(2):
                            nc.tensor.matmul(
                                pso0s[c2],
                                v_sb[:, kt, 0, :],
                                pT[:, bass.ts(c2, 512)],
                                start=(kt == 0), stop=(kt == NKT - 1),
                            )
                for c2 in range(2):
                    rcp_sb = pT_pool.tile([65, 512], fm, name="rcp_sb", tag="rcp", bufs=2)
                    with nc.allow_low_precision(reason="fp32r storage"):
                        nc.vector.reciprocal(out=rcp_sb[64:65, :], in_=pso0s[c2][64:65, :])
                    nc.sync.dma_start(
                        out=r6_tiles[0][0:1, bass.ts(c2, 512)], in_=rcp_sb[64:65, :])
                    nc.vector.tensor_copy(
                        out=oT_sb[0:64, 0, bass.ts(c2, 512)], in_=pso0s[c2][0:64, :])
                attend(0, qhs=[1])
                proj_qk(1, xt_sb, wq_sb, wk_sb)
                attend(1)
                normalize(0)
                for _q in range(QH):
                    attend(2, qhs=[_q])
                    attend(3, qhs=[_q])
                proj_qk(2, xt_sb, wq_sb, wk_sb)
                normalize(1)

            # x / qkv weights released: run the dt<2 part of the output
            # projection under the last pair's attention
            with tc.tile_pool(name="y01", bufs=1) as y01p, \
                 tc.tile_pool(name="yp", bufs=2) as ypool:
                wp_sb = ypool.tile([128, 3, EMBED], fm, name="wp_sb", bufs=1)
                nc.sync.dma_start(out=wp_sb, in_=wp_d.ap().rearrange("(n p) m -> p n m", p=128))
                y01_sb = y01p.tile([128, NQT, EMBED], f32, name="y01_sb")

                def proj01(qts):
                    for qt in qts:
                        for nh in range(2):
                            psy = ps_pool.tile([128, 512], f32, name="psy", tag="ps")
                            for dt in range(2):
                                nc.tensor.matmul(
                                    psy[:, 0:384],
                                    oT_sb[:, dt, bass.ts(qt, 128)],
                                    wp_sb[:, dt, bass.ts(nh, 384)],
                                    start=(dt == 0), stop=(dt == 1),
                                )
                            nc.vector.tensor_copy(
                                out=y01_sb[:, qt, bass.ts(nh, 384)],
                                in_=psy[:, 0:384],
                            )

                for _q in range(QH):
                    attend(4, qhs=[_q])
                    attend(5, qhs=[_q])
                    proj01(range(NQT // QH * _q, NQT // QH * (_q + 1)))

                # tail: per 512-wide chunk, normalize pair 2 then finish the
                # dt=2 projection in place and ship the output chunk
                r6p2 = r6_tiles[2]
                for c in range(4):
                    csl = bass.ts(c, 512)
                    psr = ps_pool.tile([128, 512], f32, name="psr", tag="ps")
                    nc.tensor.matmul(psr, e2_sb, r6p2[:, csl], start=True, stop=True)
                    nc.vector.tensor_mul(
                        out=oT_sb[:, 2, csl], in0=oT_sb[:, 2, csl], in1=psr,
                    )
                    for qt in range(4 * c, 4 * c + 4):
                        for nh in range(2):
                            psy2 = ps_pool.tile([128, 512], f32, name="psy2", tag="ps")
                            nc.tensor.matmul(
                                psy2[:, 0:384],
                                oT_sb[:, 2, bass.ts(qt, 128)],
                                wp_sb[:, 2, bass.ts(nh, 384)],
                                start=True, stop=True,
                            )
                            nc.vector.tensor_add(
                                out=y01_sb[:, qt, bass.ts(nh, 384)],
                                in0=y01_sb[:, qt, bass.ts(nh, 384)],
                                in1=psy2[:, 0:384],
                            )
                    for c2 in range(4):
                        cc = 4 * c + c2
                        nc.sync.dma_start(
                            out=y_d.ap()[bass.ds(128 * cc, 128), :],
                            in_=y01_sb[:, cc, :],
                        )

    nc.finalize()
    return nc


def _shard_inputs(x, w_qkv, b_qkv, w_proj):
    e2 = np.zeros((2, 128), dtype=np.float32)
    e2[0, 0:HD] = 1.0
    e2[1, HD:128] = 1.0
    in_maps = []
    for c in range(NCORES):
        b, g = c // 2, c % 2
        sl = slice(DL * g, DL * g + DL)
        in_maps.append({
            "xt": np.ascontiguousarray(x[b].T),
            "wq": np.ascontiguousarray(w_qkv[:, sl]),
            "wk": np.ascontiguousarray(w_qkv[:, EMBED:][:, sl]),
            "wv": np.ascontiguousarray(w_qkv[:, 2 * EMBED:][:, sl]),
            "bqs": np.ascontiguousarray(b_qkv[sl]),
            "bk": np.ascontiguousarray(b_qkv[EMBED:][sl]),
            "wp": np.ascontiguousarray(w_proj[sl, :]),
            "e2": e2,
        })
    return in_maps


def kernel(x, w_qkv, b_qkv, w_proj, b_proj, _profile=False, _repeat=1):
    from concourse.bass_utils import run_bass_kernel_spmd

    x = np.asarray(x, dtype=np.float32)
    w_qkv = np.asarray(w_qkv, dtype=np.float32)
    b_qkv = np.asarray(b_qkv, dtype=np.float32)
    w_proj = np.asarray(w_proj, dtype=np.float32)
    b_proj = np.asarray(b_proj, dtype=np.float32)

    if _repeat not in _prog_cache:
        _prog_cache[_repeat] = _build_program(_repeat)
    nc = _prog_cache[_repeat]

    in_maps = _shard_inputs(x, w_qkv, b_qkv, w_proj)
    res = run_bass_kernel_spmd(
        nc, in_maps, list(range(NCORES)), trace=_profile,
    )

    # host-side gather: sum the two head-group partials per batch and add
    # the bias row (v-bias folded through w_proj, plus b_proj itself)
    bias_row = b_qkv[2 * EMBED:] @ w_proj + b_proj
    y = np.empty((B, T, EMBED), dtype=np.float32)
    for b in range(B):
        y[b] = res.results[2 * b]["y"] + res.results[2 * b + 1]["y"] + bias_row
    if _profile:
        return y, res
    return y

